# revision 10
# baseline (speedup 1.0000x reference)
"""Trainium2 Bass kernel for the CustomRNNDecoder (GRU decoder) problem.

Data-parallel over batch: 8 NeuronCores x 8 batch rows each. Everything on
device is kept "D-major" (hidden/gate dim on partitions, batch on the free
dim) so the sequential GRU scan needs no transposes:

  - gh.T[gate_chunk, b] = sum_k w_hh.T-tile[k, gate_chunk].T @ h.T[k, b]
    (stationary = weight tile [128, 128] bf16, moving = h [128, 8] bf16)
  - gate math on Vector/Scalar engines on [128, 4, 8] tiles (free dim 32)
  - input-side projection Gi = x @ w_ih.T + biases precomputed in bulk
  - output projection lin_w @ h.T done in bulk after the scan

The scan over the 257 steps is emitted as a tc.For_i HARDWARE loop (the body
is ~60 instructions with register-offset APs into the Gi / h-history SBUF
tensors) instead of fully unrolled; program size is ~500 instructions
instead of ~16.6k, which removes the per-emitted-instruction overhead that
dominated the unrolled version in this execution environment.

Host side does only the embedding gather + layout shuffles (pure memcpy-type
work); all FLOPs run on the NeuronCores.
"""

import os
import sys

import numpy as np

sys.path.insert(0, "/opt/trn_rl_repo")

import ml_dtypes

BF16 = ml_dtypes.bfloat16

# Problem constants (hardcoded per the harness contract).
B, U, V, D, J = 64, 256, 32000, 512, 640
T = U + 1            # 257 scan steps
NCORES = 8
BL = B // NCORES     # 8 batch rows per core
KD = D // 128        # 4 contraction chunks
G3 = (3 * D) // 128  # 12 gate chunks
JC = J // 128        # 5 output chunks
NTOK = T * BL        # 2056 (token, batch) pairs per core
BOS = 0

# Token blocks for bulk matmuls: blocks of 64 time steps (= 512 free cols).
TBLK = 64

# Scan steps per For_i iteration (amortizes the per-iteration all-engine
# barrier + semaphore-reset overhead; leftover steps are emitted unrolled
# after the loop).
SCAN_UNROLL = int(os.environ.get("KERNEL_SCAN_UNROLL", "1"))

_PROG_CACHE = {}


def _build_program(t_steps, repeat=1, repeat_phases=("gi", "scan", "out")):
    import concourse.bass as bass
    import concourse.tile as tile
    from concourse import bacc, mybir

    f32 = mybir.dt.float32
    bf16 = mybir.dt.bfloat16
    ds = bass.ds
    AF = mybir.ActivationFunctionType

    ntok = t_steps * BL
    tblocks = [
        (i * TBLK, min(TBLK, t_steps - i * TBLK))
        for i in range((t_steps + TBLK - 1) // TBLK)
    ]

    nc = bacc.Bacc(
        "TRN2",
        target_bir_lowering=False,
        debug=False,
        enable_asserts=True,
        num_devices=1,
    )

    xt_d = nc.dram_tensor("xt", [128, KD, ntok], bf16, kind="ExternalInput").ap()
    wih_d = nc.dram_tensor("wih", [128, KD, G3, 128], bf16, kind="ExternalInput").ap()
    whh_d = nc.dram_tensor("whh", [128, KD, G3, 128], bf16, kind="ExternalInput").ap()
    lin_d = nc.dram_tensor("lin", [128, KD, JC, 128], bf16, kind="ExternalInput").ap()
    brz_d = nc.dram_tensor("brz", [128, 8], f32, kind="ExternalInput").ap()
    bni_d = nc.dram_tensor("bni", [128, KD], f32, kind="ExternalInput").ap()
    bhn_d = nc.dram_tensor("bhn", [128, KD, BL], f32, kind="ExternalInput").ap()
    linb_d = nc.dram_tensor("linb", [128, JC], f32, kind="ExternalInput").ap()
    h0_d = nc.dram_tensor("h0", [128, KD, BL], bf16, kind="ExternalInput").ap()
    outp_d = nc.dram_tensor("outp", [128, JC, ntok], f32, kind="ExternalOutput").ap()

    with tile.TileContext(nc) as tc:
        with tc.tile_pool(name="const", bufs=1) as constp:
            # Load everything resident into SBUF.
            xt = constp.tile([128, KD, ntok], bf16)
            nc.sync.dma_start(xt[:], xt_d[:])
            wih = constp.tile([128, KD, G3, 128], bf16)
            nc.sync.dma_start(wih[:], wih_d[:])
            whh = constp.tile([128, KD, G3, 128], bf16)
            nc.sync.dma_start(whh[:], whh_d[:])
            lin = constp.tile([128, KD, JC, 128], bf16)
            nc.sync.dma_start(lin[:], lin_d[:])
            brz = constp.tile([128, 8], f32)
            nc.sync.dma_start(brz[:], brz_d[:])
            bni = constp.tile([128, KD], f32)
            nc.sync.dma_start(bni[:], bni_d[:])
            bhn = constp.tile([128, KD, BL], f32)
            nc.sync.dma_start(bhn[:], bhn_d[:])
            linb = constp.tile([128, JC], f32)
            nc.sync.dma_start(linb[:], linb_d[:])

            # Big persistent buffers.
            git = constp.tile([128, G3, ntok], bf16)            # input-side gates
            # h history, flat time*batch axis; slot 0 = h0, step t writes
            # slot t+1.
            hall = constp.tile([128, KD, (t_steps + 1) * BL], bf16)
            nc.sync.dma_start(hall[:, :, 0:BL], h0_d[:])

            def emit_phase1_block(gips, boff, sz):
                # Gi = x @ w_ih.T (+ biases) for one token block.
                # boff may be a loop var (RuntimeValue) or python int.
                for g in range(G3):
                    bias_ap = brz[:, g : g + 1] if g < 8 else bni[:, g - 8 : g - 7]
                    ps = gips.tile([128, TBLK * BL], f32, tag="gi")
                    for k in range(KD):
                        nc.tensor.matmul(
                            ps[:, :sz],
                            wih[:, k, g, :],
                            xt[:, k, ds(boff, sz)],
                            start=(k == 0),
                            stop=(k == KD - 1),
                        )
                    nc.vector.tensor_scalar(
                        git[:, g, ds(boff, sz)],
                        ps[:, :sz],
                        bias_ap,
                        None,
                        mybir.AluOpType.add,
                    )

            def emit_scan_step(t, scanps, ew):
                # t may be a python int (unrolled) or a For_i loop var
                # (RuntimeValue); all h/gi accesses go through dynamic
                # slices so both work.
                toff = t * BL
                ps_r = scanps.tile([128, KD, BL], f32, tag="ps_r")
                ps_z = scanps.tile([128, KD, BL], f32, tag="ps_z")
                ps_n = scanps.tile([128, KD, BL], f32, tag="ps_n")
                # Matmul order r, n, z: the long n-gate elementwise chain
                # (t1a..tanh..d) overlaps the z-group matmuls, leaving only
                # the short z tail (pre_z' -> zc -> e -> h') on the critical
                # path before the next step can start.
                for gg, ps in ((0, ps_r), (2, ps_n), (1, ps_z)):
                    for j in range(KD):
                        g = gg * KD + j
                        for k in range(KD):
                            nc.tensor.matmul(
                                ps[:, j, :],
                                whh[:, k, g, :],
                                hall[:, k, ds(toff, BL)],
                                start=(k == 0),
                                stop=(k == KD - 1),
                            )
                # Gate math (fp32 intermediates; h stored bf16).
                gi_r = git[:, 0:KD, ds(toff, BL)]
                gi_z = git[:, KD : 2 * KD, ds(toff, BL)]
                gi_n = git[:, 2 * KD : 3 * KD, ds(toff, BL)]
                h_prev = hall[:, :, ds(toff, BL)]

                pre_r = ew.tile([128, KD, BL], f32, tag="pre_r")
                nc.vector.tensor_add(pre_r[:], ps_r[:], gi_r)
                r = ew.tile([128, KD, BL], f32, tag="r")
                nc.scalar.activation(r[:], pre_r[:], AF.Sigmoid)

                t1a = ew.tile([128, KD, BL], f32, tag="t1a")
                nc.vector.tensor_add(t1a[:], ps_n[:], bhn[:])
                t1 = ew.tile([128, KD, BL], f32, tag="t1")
                nc.vector.tensor_mul(t1[:], t1a[:], r[:])
                t2 = ew.tile([128, KD, BL], f32, tag="t2")
                nc.vector.tensor_add(t2[:], t1[:], gi_n)
                n_g = ew.tile([128, KD, BL], f32, tag="n_g")
                nc.scalar.activation(n_g[:], t2[:], AF.Tanh)
                d_g = ew.tile([128, KD, BL], f32, tag="d_g")
                nc.vector.tensor_sub(d_g[:], n_g[:], h_prev)

                # pre_z' = -(ps_z + gi_z) fused in one op; zc = sigmoid(pre_z')
                pre_z = ew.tile([128, KD, BL], f32, tag="pre_z")
                nc.vector.scalar_tensor_tensor(
                    pre_z[:], ps_z[:], -1.0, gi_z,
                    mybir.AluOpType.mult, mybir.AluOpType.subtract,
                )
                zc = ew.tile([128, KD, BL], f32, tag="zc")
                nc.scalar.activation(zc[:], pre_z[:], AF.Sigmoid)

                e_g = ew.tile([128, KD, BL], f32, tag="e_g")
                nc.vector.tensor_mul(e_g[:], zc[:], d_g[:])
                nc.vector.tensor_add(hall[:, :, ds(toff + BL, BL)], h_prev, e_g[:])

            def emit_phase3_block(ops, oevac, boff, sz):
                # out = h @ lin_w.T + lin_b for one token block, then DMA out.
                for c in range(JC):
                    ps = ops.tile([128, TBLK * BL], f32, tag="op")
                    for k in range(KD):
                        nc.tensor.matmul(
                            ps[:, :sz],
                            lin[:, k, c, :],
                            hall[:, k, ds(boff + BL, sz)],
                            start=(k == 0),
                            stop=(k == KD - 1),
                        )
                    ot = oevac.tile([128, TBLK * BL], f32, tag="ot")
                    nc.vector.tensor_scalar(
                        ot[:, :sz],
                        ps[:, :sz],
                        linb[:, c : c + 1],
                        None,
                        mybir.AluOpType.add,
                    )
                    nc.sync.dma_start(outp_d[:, c, ds(boff, sz)], ot[:, :sz])

            nfull = t_steps // TBLK          # full 64-step token blocks
            tail0, tailn = nfull * TBLK, t_steps - nfull * TBLK

            uid = [0]

            def _nm(base):
                uid[0] += 1
                return f"{base}{uid[0]}"

            def emit_gi():
                with tc.tile_pool(name=_nm("gips"), bufs=2, space="PSUM") as gips:
                    if nfull:
                        with tc.For_i(0, nfull) as bi:
                            emit_phase1_block(gips, bi * (TBLK * BL), TBLK * BL)
                    if tailn:
                        emit_phase1_block(gips, tail0 * BL, tailn * BL)

            def emit_scan():
                su = max(1, SCAN_UNROLL)
                nloop = t_steps // su * su       # steps covered by the loop
                sbufs = 1 if su == 1 else 2
                with tc.tile_pool(name=_nm("scanps"), bufs=sbufs, space="PSUM") as scanps, \
                     tc.tile_pool(name=_nm("ew"), bufs=sbufs) as ew:
                    if nloop:
                        with tc.For_i(0, nloop, su) as t:
                            for s in range(su):
                                emit_scan_step(t + s if s else t, scanps, ew)
                    for ts in range(nloop, t_steps):
                        emit_scan_step(ts, scanps, ew)

            def emit_out():
                with tc.tile_pool(name=_nm("ops"), bufs=2, space="PSUM") as ops, \
                     tc.tile_pool(name=_nm("oevac"), bufs=3) as oevac:
                    if nfull:
                        with tc.For_i(0, nfull) as bi:
                            emit_phase3_block(ops, oevac, bi * (TBLK * BL), TBLK * BL)
                    if tailn:
                        emit_phase3_block(ops, oevac, tail0 * BL, tailn * BL)

            def emit_scanpe():
                # Probe: matmul side of the scan only (static h operand, no
                # feedback, psum evacuated to sinks).  Output-neutral.
                with tc.tile_pool(name=_nm("pps"), bufs=2, space="PSUM") as pps, \
                     tc.tile_pool(name=_nm("sink"), bufs=2) as sinkp:
                    with tc.For_i(0, t_steps) as t:
                        pss = []
                        for nm, gg in (("r", 0), ("n", 2), ("z", 1)):
                            ps = pps.tile([128, KD, BL], f32, tag=f"pp_{nm}")
                            pss.append(ps)
                            for j in range(KD):
                                g = gg * KD + j
                                for k in range(KD):
                                    nc.tensor.matmul(
                                        ps[:, j, :],
                                        whh[:, k, g, :],
                                        hall[:, k, 0:BL],
                                        start=(k == 0),
                                        stop=(k == KD - 1),
                                    )
                        for i, ps in enumerate(pss):
                            sink = sinkp.tile([128, KD, BL], f32, tag=f"sk{i}")
                            nc.scalar.copy(sink[:], ps[:])

            def emit_scanew():
                # Probe: elementwise side of the scan only (reads a zeroed
                # sbuf tile in place of the psums, writes sinks).
                with tc.tile_pool(name=_nm("ewp"), bufs=1) as ewp:
                    zz = ewp.tile([128, KD, BL], f32, tag="zz")
                    nc.vector.memset(zz[:], 0.0)
                    with tc.For_i(0, t_steps) as t:
                        toff = t * BL
                        gi_r = git[:, 0:KD, ds(toff, BL)]
                        gi_z = git[:, KD : 2 * KD, ds(toff, BL)]
                        gi_n = git[:, 2 * KD : 3 * KD, ds(toff, BL)]
                        h_prev = hall[:, :, ds(toff, BL)]
                        pre_r = ewp.tile([128, KD, BL], f32, tag="pre_r")
                        nc.vector.tensor_add(pre_r[:], zz[:], gi_r)
                        r = ewp.tile([128, KD, BL], f32, tag="r")
                        nc.scalar.activation(r[:], pre_r[:], AF.Sigmoid)
                        t1a = ewp.tile([128, KD, BL], f32, tag="t1a")
                        nc.vector.tensor_add(t1a[:], zz[:], bhn[:])
                        t1 = ewp.tile([128, KD, BL], f32, tag="t1")
                        nc.vector.tensor_mul(t1[:], t1a[:], r[:])
                        t2 = ewp.tile([128, KD, BL], f32, tag="t2")
                        nc.vector.tensor_add(t2[:], t1[:], gi_n)
                        n_g = ewp.tile([128, KD, BL], f32, tag="n_g")
                        nc.scalar.activation(n_g[:], t2[:], AF.Tanh)
                        d_g = ewp.tile([128, KD, BL], f32, tag="d_g")
                        nc.vector.tensor_sub(d_g[:], n_g[:], h_prev)
                        pre_z = ewp.tile([128, KD, BL], f32, tag="pre_z")
                        nc.vector.scalar_tensor_tensor(
                            pre_z[:], zz[:], -1.0, gi_z,
                            mybir.AluOpType.mult, mybir.AluOpType.subtract,
                        )
                        zc = ewp.tile([128, KD, BL], f32, tag="zc")
                        nc.scalar.activation(zc[:], pre_z[:], AF.Sigmoid)
                        e_g = ewp.tile([128, KD, BL], f32, tag="e_g")
                        nc.vector.tensor_mul(e_g[:], zc[:], d_g[:])
                        sink = ewp.tile([128, KD, BL], f32, tag="snk")
                        nc.vector.tensor_add(sink[:], h_prev, e_g[:])

            probes = {
                "gi": emit_gi, "scan": emit_scan, "out": emit_out,
                "scanpe": emit_scanpe, "scanew": emit_scanew,
            }

            # The correct compute body always runs once; repeat > 1 then
            # re-runs the phases in repeat_phases (repeat-1) times via an
            # ON-DEVICE loop, so program size is repeat-invariant and wall
            # deltas isolate body execution time.  All probe bodies are
            # output-neutral.
            emit_gi()
            emit_scan()
            emit_out()
            if repeat > 1:
                with tc.For_i(0, repeat - 1) as _rep:
                    for ph in repeat_phases:
                        probes[ph]()

    nc.compile()
    return nc


def _get_program(t_steps, repeat=1, repeat_phases=("gi", "scan", "out")):
    key = (t_steps, repeat, tuple(repeat_phases))
    if key not in _PROG_CACHE:
        _PROG_CACHE[key] = _build_program(t_steps, repeat, repeat_phases)
    return _PROG_CACHE[key]


def kernel(src_tokens, src_lengths, embed_w, w_ih, w_hh, b_ih, b_hh, lin_w, lin_b,
           init_state, _t_steps=T, _want_results=False, _trace=False, _tmpdir=None,
           _repeat=1, _repeat_phases=("gi", "scan", "out"), **_ignored):
    from concourse.bass_utils import run_bass_kernel_spmd

    src_tokens = np.asarray(src_tokens)
    embed_w = np.asarray(embed_w, dtype=np.float32)
    w_ih = np.asarray(w_ih, dtype=np.float32)
    w_hh = np.asarray(w_hh, dtype=np.float32)
    b_ih = np.asarray(b_ih, dtype=np.float32)
    b_hh = np.asarray(b_hh, dtype=np.float32)
    lin_w = np.asarray(lin_w, dtype=np.float32)
    lin_b = np.asarray(lin_b, dtype=np.float32)
    init_state = np.asarray(init_state, dtype=np.float32)

    t_steps = _t_steps
    ntok = t_steps * BL

    # Host prep: embedding gather + layout shuffles (no FLOPs).
    tokens = np.concatenate(
        [np.full((B, 1), BOS, dtype=src_tokens.dtype), src_tokens], axis=1
    )[:, :t_steps]                                   # [B, T]
    X = embed_w[tokens].astype(BF16)                 # [B, T, D]

    def dmaj(vec):  # [D] -> [128, KD]
        return np.ascontiguousarray(vec.reshape(KD, 128).T)

    wih_t = np.ascontiguousarray(
        w_ih.reshape(G3, 128, KD, 128).transpose(3, 2, 0, 1)).astype(BF16)
    whh_t = np.ascontiguousarray(
        w_hh.reshape(G3, 128, KD, 128).transpose(3, 2, 0, 1)).astype(BF16)
    lin_t = np.ascontiguousarray(
        lin_w.reshape(JC, 128, KD, 128).transpose(3, 2, 0, 1)).astype(BF16)
    brz = np.ascontiguousarray((b_ih + b_hh)[: 2 * D].reshape(8, 128).T)
    bni = dmaj(b_ih[2 * D :])
    bhn = np.ascontiguousarray(
        np.broadcast_to(dmaj(b_hh[2 * D :])[:, :, None], (128, KD, BL)))
    linb = np.ascontiguousarray(lin_b.reshape(JC, 128).T)
    h0 = np.ascontiguousarray(
        np.broadcast_to(dmaj(init_state)[:, :, None], (128, KD, BL))).astype(BF16)

    shared = {
        "wih": wih_t, "whh": whh_t, "lin": lin_t,
        "brz": brz.astype(np.float32), "bni": bni.astype(np.float32),
        "bhn": bhn.astype(np.float32), "linb": linb.astype(np.float32),
        "h0": h0,
    }
    in_maps = []
    for c in range(NCORES):
        xc = X[c * BL : (c + 1) * BL]                # [BL, T, D]
        xt = np.ascontiguousarray(
            xc.reshape(BL, t_steps, KD, 128).transpose(3, 2, 1, 0)
        ).reshape(128, KD, ntok)
        in_maps.append({**shared, "xt": np.ascontiguousarray(xt)})

    nc = _get_program(t_steps, _repeat, _repeat_phases)
    res = run_bass_kernel_spmd(
        nc, in_maps, core_ids=list(range(NCORES)), trace=_trace, tmpdir=_tmpdir
    )

    out = np.empty((B, t_steps, J), dtype=np.float32)
    for c in range(NCORES):
        o = res.results[c]["outp"]                   # [128, JC, ntok]
        o = o.reshape(128, JC, t_steps, BL).transpose(3, 2, 1, 0)  # [BL,T,JC,128]
        out[c * BL : (c + 1) * BL] = o.reshape(BL, t_steps, J)
    if _want_results:
        return out, res
    return out


if __name__ == "__main__":
    # Quick smoke test with a tiny number of steps.
    t_steps = int(os.environ.get("KERNEL_T", "8"))
    rng = np.random.default_rng(0)
    ins = {
        "src_tokens": rng.integers(0, V, size=(B, U)).astype(np.int64),
        "src_lengths": rng.integers(1, U, size=(B,)).astype(np.int32),
        "embed_w": (rng.standard_normal((V, D)) * 0.02).astype(np.float32),
        "w_ih": (rng.standard_normal((3 * D, D)) / np.sqrt(D)).astype(np.float32),
        "w_hh": (rng.standard_normal((3 * D, D)) / np.sqrt(D)).astype(np.float32),
        "b_ih": (rng.standard_normal(3 * D) * 0.01).astype(np.float32),
        "b_hh": (rng.standard_normal(3 * D) * 0.01).astype(np.float32),
        "lin_w": (rng.standard_normal((J, D)) / np.sqrt(D)).astype(np.float32),
        "lin_b": (rng.standard_normal(J) * 0.01).astype(np.float32),
        "init_state": rng.standard_normal(D).astype(np.float32),
    }
    actual = kernel(**ins, _t_steps=t_steps)

    # numpy reference for t_steps
    tokens = np.concatenate(
        [np.zeros((B, 1), dtype=np.int64), ins["src_tokens"]], axis=1)[:, :t_steps]
    x_all = ins["embed_w"][tokens]
    h = np.broadcast_to(ins["init_state"], (B, D)).astype(np.float32)
    outs = []
    for t in range(t_steps):
        gi = x_all[:, t] @ ins["w_ih"].T + ins["b_ih"]
        gh = h @ ins["w_hh"].T + ins["b_hh"]
        i_r, i_z, i_n = np.split(gi, 3, axis=-1)
        h_r, h_z, h_n = np.split(gh, 3, axis=-1)
        r = 1 / (1 + np.exp(-(i_r + h_r)))
        z = 1 / (1 + np.exp(-(i_z + h_z)))
        n = np.tanh(i_n + r * h_n)
        h = (1 - z) * n + z * h
        outs.append(h @ ins["lin_w"].T + ins["lin_b"])
    expected = np.stack(outs, axis=1)
    err = np.abs(actual - expected)
    rel = np.linalg.norm(actual - expected) / np.linalg.norm(expected)
    print("max abs err:", err.max(), "rel l2:", rel)
